# revision 22
# baseline (speedup 1.0000x reference)
"""Trainium2 Bass kernel for nn_Decoder_84997402788281.

Data-parallel over batch B=64 across 8 NeuronCores (8 rows each).
Heavy matmuls in fp8e4 with DoubleRow perf mode (two k-tiles fused per
matmul at 0.5 cycles/col); softmax/gates/output fp32.  Small-magnitude
weights are pre-scaled by WS=32 on host and descaled on-chip (activation
scale or tensor_scalar); v vectors by VS=64, descaled at the exp.

Layout conventions (per core, local batch b in [0,8)):
  - "img" tensors are SBUF partition images: a [K, N] matrix padded to
    KT*128 rows is shipped as [128, KT*N] with img[p, k*N+n] = M[k*128+p, n].
    KT is kept even so pairs of k-tiles feed DoubleRow matmuls.
  - Activations for attention are pre-transposed on host: memT[b] = mem_b.T
    with shape [H, L], shipped as per-b fp8 images [128, 8*L].
  - Biases are folded into matmuls via an appended ones-row (aug) tile.
  - Extended vocab (V=32000 + 50 oov, padded to 32768) lives in a per-b grid
    [128 partitions, 256 free] with v = p*256 + f.  The pointer scatter-add is
    an exact one-hot matmul: grid[p,f] += sum_j [hi_j==p][lo_j==f] * attn_j,
    hi = idx//256, lo = idx%256 (host precomputed, bf16 — ints <= 300 exact).
  - Logits round-trip DRAM in fp16 to transpose [b, v] -> per-b grids.
  - w2 chunks stream on the SP DMA queue; ~26 chunks are prefetched,
    interleaved with the attention b-loop, so the out2 phase is not
    DMA-serialized at the end.
"""

import os
import numpy as np
import ml_dtypes

import concourse.bass as bass
import concourse.tile as tile
from concourse import bacc, mybir
from concourse.bass_utils import run_bass_kernel_spmd
from concourse.masks import make_identity

BF16 = ml_dtypes.bfloat16
FP8 = ml_dtypes.float8_e4m3
F32 = np.float32

# Problem dims (hardcoded per contract)
B, S, LC, LU, LG = 64, 8, 64, 512, 64
H, E, V, OOV = 1024, 512, 32000, 50
NCORES = 8
BL = B // NCORES            # 8 local batch rows
VP = 32768                  # padded vocab = 128 * 256
VEXT = V + OOV              # 32050
NJ = 1152                   # padded pointer-index count = 9*128 (1088 real)
NJ_REAL = LU + S * LC + LG  # 1088
NKJ = NJ // 128             # 9
NVC = VP // 512             # 64 v-chunks of 512
NT = NVC // 4               # 16 groups of 4 col-packed chunks

WS = 32.0                   # weight pre-scale for fp8
VS = 64.0                   # v-vector pre-scale for fp8
N_PREF = 22                 # w2 chunks prefetched during attention

dtb = mybir.dt.bfloat16
dtf = mybir.dt.float32
dt8 = mybir.dt.float8e4
dt16 = mybir.dt.float16
AF = mybir.ActivationFunctionType
OP = mybir.AluOpType
DR = mybir.MatmulPerfMode.DoubleRow

_CACHE = {}
LAST_RESULTS = None


# ---------------------------------------------------------------- host prep

def _img_cast(M, kt, dtype, scale=1.0):
    """[K, N] float array -> [128, kt*N] partition image (K padded)."""
    M = np.asarray(M, F32) * scale
    K, N = M.shape
    Mp = np.zeros((kt * 128, N), F32)
    Mp[:K] = M
    return np.ascontiguousarray(
        Mp.reshape(kt, 128, N).transpose(1, 0, 2).reshape(128, kt * N)
    ).astype(dtype)


def _memT_img8(mem, residual=False):
    """[BL, L, H] -> [BL, 128, 8*L] fp8 (memT per-b image).

    residual=True returns fp8(x - fp8(x)) in the same layout (the second
    DoubleRow compensation pass)."""
    BLc, L, Hc = mem.shape
    assert Hc == H
    t = np.ascontiguousarray(np.asarray(mem, F32).transpose(0, 2, 1))  # [BL,H,L]
    t = t.reshape(BLc, 8, 128, L).transpose(0, 2, 1, 3).reshape(BLc, 128, 8 * L)
    t = np.ascontiguousarray(t)
    if residual:
        t = t - t.astype(FP8).astype(F32)
    return t.astype(FP8)


def _resid8(img_f32_scaled):
    """fp8 residual of an already-scaled f32 image."""
    lo = img_f32_scaled - img_f32_scaled.astype(FP8).astype(F32)
    return lo.astype(FP8)


def _prep_inputs(inp):
    """Build per-core input maps (list of dicts of np arrays)."""
    g = {k: np.asarray(v) for k, v in inp.items()}
    assert int(g["oovs_max"]) == OOV

    emb = np.asarray(g["input_emb"], F32)                 # [B, E]
    ctx0 = np.asarray(g["dec_init_context"], F32)[:, 0, :]  # [B, H]
    h0 = np.asarray(g["dec_hidden"], F32)[-1]             # [B, H]

    x = np.concatenate([emb, ctx0], axis=1)               # [B, E+H]
    wihT = np.concatenate(
        [np.asarray(g["gru_w_ih"], F32).T, np.asarray(g["gru_b_ih"], F32)[None, :]], 0
    )  # [1537, 3072]
    whhT = np.concatenate(
        [np.asarray(g["gru_w_hh"], F32).T, np.asarray(g["gru_b_hh"], F32)[None, :]], 0
    )  # [1025, 3072]

    wq_imgs, wm_imgs, wmr_imgs, v_cols = [], [], [], []
    for name in ["utt", "cue", "goal", "hl"]:
        wqT = np.concatenate(
            [np.asarray(g[name + "_Wq"], F32).T, np.asarray(g[name + "_bq"], F32)[None, :]], 0
        )  # [1025, 1024]
        wq_imgs.append(_img_cast(wqT, 9, FP8, WS))
        wm_imgs.append(_img_cast(np.asarray(g[name + "_Wm"], F32).T, 8, FP8, WS))
        wmr_imgs.append(_resid8(_img_cast(np.asarray(g[name + "_Wm"], F32).T, 8, F32, WS)))
        v_cols.append(np.asarray(g[name + "_v"], F32).reshape(8, 128).T)
    v_imgb = np.ascontiguousarray(np.concatenate(v_cols, axis=1)).astype(BF16)
    v_img = np.ascontiguousarray(VS * np.concatenate(v_cols, axis=1)).astype(FP8)  # [128,32]

    out1T = np.concatenate(
        [np.asarray(g["out1_W"], F32).T, np.asarray(g["out1_b"], F32)[None, :]], 0
    )  # [2049, 1024]
    pgenT = np.concatenate(
        [np.asarray(g["pgen_W"], F32).T, np.asarray(g["pgen_b"], F32)[None, :]], 0
    )  # [2561, 1]

    # out2: [V, H] -> transposed padded [1024, VP] fp8 (x WS); bias separate
    w2T = np.zeros((H, VP), F32)
    w2T[:, :V] = WS * np.asarray(g["out2_W"], F32).T
    w2_img = (
        w2T.astype(FP8)
        .reshape(8, 128, NVC, 512)
        .transpose(2, 1, 0, 3)
        .reshape(NVC, 128, 8 * 512)
    )
    w2_img = np.ascontiguousarray(w2_img)  # [64, 128, 4096] fp8
    bias_pad = np.full((VP,), -64.0, F32)
    bias_pad[:V] = np.asarray(g["out2_b"], F32)
    w2b = np.ascontiguousarray(bias_pad.reshape(128, 256))

    # pointer indices
    idx = np.concatenate(
        [
            np.asarray(g["valid_src_extend_vocab"]).astype(np.int64),
            np.asarray(g["valid_cue_extend_vocab"]).astype(np.int64).reshape(B, -1),
            np.asarray(g["valid_goal_extend_vocab"]).astype(np.int64),
        ],
        axis=1,
    )  # [B, 1088]
    assert idx.min() >= 0 and idx.max() < VEXT
    hi = np.full((B, NJ), 300.0, F32)
    lo = np.full((B, NJ), 300.0, F32)
    hi[:, :NJ_REAL] = (idx // 256).astype(F32)
    lo[:, :NJ_REAL] = (idx % 256).astype(F32)

    # masks -> additive -1e9 rows, flattened (b-major)
    m_utt = (-1e9 * np.asarray(g["utt_attn_mask"], F32)).astype(BF16)      # [B,512]
    m_cue = (-1e9 * np.asarray(g["cue_attn_mask"], F32)).astype(BF16).reshape(B, S * LC)
    m_goal = (-1e9 * np.asarray(g["goal_attn_mask"], F32)).astype(BF16)    # [B,64]
    kg = np.asarray(g["kg_attn_dist"], F32)                                # [B,S]

    utt = np.asarray(g["utt_outputs"], F32)
    cue = np.asarray(g["cue_outputs"], F32).reshape(B, S * LC, H)
    goal = np.asarray(g["goal_outputs"], F32)

    in_maps = []
    for c in range(NCORES):
        sl = slice(c * BL, (c + 1) * BL)
        xT = np.zeros((1537, 16), F32)
        xT[:1536, :BL] = x[sl].T
        xT[1536, :BL] = 1.0
        h0T = np.zeros((1025, 16), F32)
        h0T[:1024, :BL] = h0[sl].T
        h0T[1024, :BL] = 1.0
        h0Tb = np.zeros((1024, 16), F32)
        h0Tb[:, :BL] = h0[sl].T
        loc = lambda a: np.ascontiguousarray(a[sl])
        lo_img = np.ascontiguousarray(
            lo[sl].reshape(BL, NKJ, 128).transpose(2, 1, 0).reshape(128, NKJ * BL)
        )
        hi_img = np.ascontiguousarray(
            hi[sl].reshape(BL, NKJ, 128).transpose(2, 1, 0).reshape(128, NKJ * BL)
        )
        m = {
            "xT": _img_cast(xT, 14, FP8),
            "wihT": _img_cast(wihT, 14, FP8, WS),
            "h0T": _img_cast(h0T, 10, FP8),
            "whhT": _img_cast(whhT, 10, FP8, WS),
            "h0Tb": _img_cast(h0Tb, 8, BF16),
            "wq0": wq_imgs[0], "wq1": wq_imgs[1], "wq2": wq_imgs[2], "wq3": wq_imgs[3],
            "wm0": wm_imgs[0], "wm1": wm_imgs[1], "wm2": wm_imgs[2], "wm3": wm_imgs[3],
            "v_img": v_img, "v_imgb": v_imgb,
            "memT_utt": _memT_img8(utt[sl]),
            "memT_cue": _memT_img8(cue[sl]),
            "memT_goal": _memT_img8(goal[sl]),
            "memR_cue": _memT_img8(cue[sl], residual=True),
            "memR_goal": _memT_img8(goal[sl], residual=True),
            "wmr1": wmr_imgs[1], "wmr2": wmr_imgs[2],
            "mask_utt": np.ascontiguousarray(m_utt[sl].reshape(1, BL * LU)),
            "mask_cue": np.ascontiguousarray(m_cue[sl].reshape(1, BL * S * LC)),
            "mask_goal": np.ascontiguousarray(m_goal[sl].reshape(1, BL * LG)),
            "kg_row": np.ascontiguousarray(kg[sl].reshape(1, BL * S)),
            "embT": _img_cast(emb[sl].T, 4, BF16),
            "out1T": _img_cast(out1T, 17, FP8, WS),
            "pgenT": _img_cast(pgenT, 21, BF16),
            "w2": w2_img,
            "w2b": w2b,
            "lo_img": lo_img,
            "hi_img": hi_img,
        }
        in_maps.append(m)
    return in_maps


# ------------------------------------------------------------- device build

def _declare(nc):
    """Declare DRAM tensors; returns dict name -> AP."""
    d = {}

    def inp(name, shape, dt):
        d[name] = nc.dram_tensor(name, list(shape), dt, kind="ExternalInput").ap()

    inp("xT", (128, 14 * 16), dt8)
    inp("wihT", (128, 14 * 3072), dt8)
    inp("h0T", (128, 10 * 16), dt8)
    inp("whhT", (128, 10 * 3072), dt8)
    inp("h0Tb", (128, 8 * 16), dtb)
    for a in range(4):
        inp(f"wq{a}", (128, 9 * 1024), dt8)
        inp(f"wm{a}", (128, 8 * 1024), dt8)
    inp("v_img", (128, 32), dt8)
    inp("v_imgb", (128, 32), dtb)
    inp("memT_utt", (BL, 128, 8 * LU), dt8)
    inp("memT_cue", (BL, 128, 8 * S * LC), dt8)
    inp("memT_goal", (BL, 128, 8 * LG), dt8)
    inp("memR_cue", (BL, 128, 8 * S * LC), dt8)
    inp("memR_goal", (BL, 128, 8 * LG), dt8)
    inp("wmr1", (128, 8 * 1024), dt8)
    inp("wmr2", (128, 8 * 1024), dt8)
    inp("mask_utt", (1, BL * LU), dtb)
    inp("mask_cue", (1, BL * S * LC), dtb)
    inp("mask_goal", (1, BL * LG), dtb)
    inp("kg_row", (1, BL * S), dtf)
    inp("embT", (128, 4 * BL), dtb)
    inp("out1T", (128, 17 * 1024), dt8)
    inp("pgenT", (128, 21 * 1), dtb)
    inp("w2", (NVC, 128, 8 * 512), dt8)
    inp("w2b", (128, 256), dtf)
    inp("lo_img", (128, NKJ * BL), dtf)
    inp("hi_img", (128, NKJ * BL), dtf)
    d["outp"] = nc.dram_tensor("outp", [BL, VP], dtf, kind="ExternalOutput").ap()
    return d


def _pairs(ap_img, kt, kp, lo_col, n_col):
    """Slice a [128, kt*N] image into the DoubleRow pair AP [128, 2, n_col]."""
    r = ap_img.rearrange("p (k n) -> p k n", k=kt)
    return r[:, 2 * kp:2 * kp + 2, lo_col:lo_col + n_col]


def _build(nc, tc, ctx, d):
    from contextlib import ExitStack

    KPH = int(os.environ.get("KPH", "99"))

    nvec, nsc, npe, ngp = nc.vector, nc.scalar, nc.tensor, nc.gpsimd

    const = ctx.enter_context(tc.tile_pool(name="const", bufs=1))
    persist = ctx.enter_context(tc.tile_pool(name="persist", bufs=1))
    o2pool = ctx.enter_context(tc.tile_pool(name="o2_w", bufs=N_PREF))

    # w2 chunk prefetch: first N_PREF chunks issue during the attention
    # b-loop; the rest are issued JIT inside the out2 loop (a blocked early
    # kick would sit on the SP queue ahead of the out1/lo/hi DMAs that out2
    # consumption depends on -> deadlock, hence the jit cap).
    o2_tiles = {}
    o2_state = {"next": 0}

    def kick_w2(n=1, jit=False):
        for _ in range(n):
            cidx = o2_state["next"]
            if cidx >= NVC or (not jit and cidx >= N_PREF):
                return
            o2_state["next"] = cidx + 1
            t = o2pool.tile([128, 8 * 512], dt8, name=f"o2_{cidx}", tag="o2")
            nc.sync.dma_start(t[:], d["w2"][cidx])
            o2_tiles[cidx] = t

    # ---- constants
    id_bf = const.tile([128, 128], dtb)
    make_identity(nc, id_bf[:])
    id_f32 = const.tile([128, 128], dtf)
    make_identity(nc, id_f32[:])
    iota128_i = const.tile([128, 128], mybir.dt.int32)
    ngp.iota(iota128_i[:], pattern=[[1, 128]], base=0, channel_multiplier=0)
    iota128 = const.tile([128, 128], dtb)
    nvec.tensor_copy(iota128[:], iota128_i[:])
    iota256_i = const.tile([128, 256], mybir.dt.int32)
    ngp.iota(iota256_i[:], pattern=[[1, 256]], base=0, channel_multiplier=0)
    iota256 = const.tile([128, 256], dtb)
    nvec.tensor_copy(iota256[:], iota256_i[:])
    ones_r_bf = const.tile([1, 128], dtb)
    ngp.memset(ones_r_bf[:], 1.0)
    ones_r_f = const.tile([1, 128], dtf)
    ngp.memset(ones_r_f[:], 1.0)
    ones_c_f = const.tile([128, 1], dtf)
    ngp.memset(ones_c_f[:], 1.0)
    v_img_t = const.tile([128, 32], dt8)
    nc.sync.dma_start(v_img_t[:], d["v_img"])
    v_bf_t = const.tile([128, 32], dtb)
    nc.sync.dma_start(v_bf_t[:], d["v_imgb"])

    # ---- persistent activations
    hT = persist.tile([128, 9 * BL], dtb)          # aug h image [1152, 8]
    qpT = [persist.tile([128, 8 * BL], dtf, name=f"qpT{a}", tag=f"qpT{a}") for a in range(4)]
    ctxT = [persist.tile([128, 8 * BL], dtf, name=f"ctxT{a}", tag=f"ctxT{a}") for a in range(3)]
    cmT = persist.tile([128, 8 * BL], dtf)         # ctx_merge T fp32
    w_utt = persist.tile([1, BL * LU], dtb)
    w_cue = persist.tile([1, BL * S * LC], dtb)    # kg * softmax
    w_goal = persist.tile([1, BL * LG], dtb)
    scat_sb = persist.tile([128, BL * 256], dtf)   # scatter grids
    pgen_row = persist.tile([1, BL], dtf)
    ompg_row = persist.tile([1, BL], dtf)
    ctw = persist.tile([1, 3 * BL], dtf)           # hl weights (b,i)
    omid32 = persist.tile([128, 8 * 128], dt8)     # rep-16 out_mid image fp8
    attnT = persist.tile([128, NKJ * BL], dtb)
    lo_t = persist.tile([128, NKJ * BL], dtf)
    hi_t = persist.tile([128, NKJ * BL], dtf)
    mask_t = {
        "utt": persist.tile([1, BL * LU], dtb, name="mku", tag="mku"),
        "cue": persist.tile([1, BL * S * LC], dtb, name="mkc", tag="mkc"),
        "goal": persist.tile([1, BL * LG], dtb, name="mkg", tag="mkg"),
    }
    nc.sync.dma_start(mask_t["utt"][:], d["mask_utt"])
    nc.sync.dma_start(mask_t["cue"][:], d["mask_cue"])
    nc.sync.dma_start(mask_t["goal"][:], d["mask_goal"])
    kg_t = persist.tile([1, BL * S], dtf)
    nc.sync.dma_start(kg_t[:], d["kg_row"])

    # ================================================================ GRU
    # W-stationary DoubleRow: giT[p=out-col-chunk m, (m,b)] = (W pair).T @ x
    # pair.  x/h0 images use width-16 k-tiles (pair stride 16B, ISA rule).
    # All gate math runs in the transposed [128, (m, b16)] layout, so h is
    # produced directly in hT layout with no PE transposes.
    with ExitStack() as ph:
        wp = ph.enter_context(tc.tile_pool(name="gru_w", bufs=1))
        tp = ph.enter_context(tc.tile_pool(name="gru_t", bufs=7))
        pp = ph.enter_context(tc.tile_pool(name="gru_ps", bufs=1, space="PSUM"))

        xT = wp.tile([128, 14 * 16], dt8)
        nc.sync.dma_start(xT[:], d["xT"])
        h0T = wp.tile([128, 10 * 16], dt8)
        nc.sync.dma_start(h0T[:], d["h0T"])
        h0Tb = wp.tile([128, 8 * 16], dtb)
        nc.sync.dma_start(h0Tb[:], d["h0Tb"])

        giT = wp.tile([128, 24 * 16], dtf)
        ghT = wp.tile([128, 24 * 16], dtf)
        for dst, rhs_img, w_dram, nk in ((giT, xT, d["wihT"], 14), (ghT, h0T, d["whhT"], 10)):
            npair = nk // 2
            wks = []
            for kp in range(npair):
                wk = tp.tile([128, 2 * 3072], dt8, name="gru_wk", tag="gru_wk")
                nc.sync.dma_start(wk[:], w_dram[:, 2 * kp * 3072:(2 * kp + 2) * 3072])
                wks.append(wk)
            ps = pp.tile([128, 24 * 16], dtf, tag="gru_ps")
            for m in range(24):
                for kp in range(npair):
                    npe.matmul(
                        ps[:, m * 16:(m + 1) * 16],
                        _pairs(wks[kp][:], 2, 0, m * 128, 128),
                        _pairs(rhs_img[:], nk, kp, 0, 16),
                        start=(kp == 0),
                        stop=(kp == npair - 1),
                        perf_mode=DR,
                    )
            nvec.tensor_copy(dst[:], ps[:])

        W16 = 8 * 16
        r = wp.tile([128, W16], dtf)
        z = wp.tile([128, W16], dtf)
        n_g = wp.tile([128, W16], dtf)
        tmp = wp.tile([128, W16], dtf)
        tmp2 = wp.tile([128, W16], dtf)
        # gi/gh are scaled by WS; descale inside the activations
        # r = sigmoid((ir+hr)/WS)
        nvec.tensor_tensor(tmp[:], giT[:, 0:W16], ghT[:, 0:W16], op=OP.add)
        nsc.activation(r[:], tmp[:], AF.Tanh, scale=0.5 / WS)
        nvec.tensor_scalar(r[:], r[:], 0.5, 0.5, op0=OP.mult, op1=OP.add)
        # z
        nvec.tensor_tensor(tmp[:], giT[:, W16:2 * W16], ghT[:, W16:2 * W16], op=OP.add)
        nsc.activation(z[:], tmp[:], AF.Tanh, scale=0.5 / WS)
        nvec.tensor_scalar(z[:], z[:], 0.5, 0.5, op0=OP.mult, op1=OP.add)
        # n = tanh((inn + r*hn)/WS)   (r unscaled, hn scaled)
        nvec.tensor_tensor(tmp[:], r[:], ghT[:, 2 * W16:3 * W16], op=OP.mult)
        nvec.tensor_tensor(tmp[:], tmp[:], giT[:, 2 * W16:3 * W16], op=OP.add)
        nsc.activation(n_g[:], tmp[:], AF.Tanh, scale=1.0 / WS)
        # h = n + z*(h0 - n)
        nvec.tensor_tensor(tmp[:], h0Tb[:], n_g[:], op=OP.subtract)
        nvec.tensor_tensor(tmp2[:], z[:], tmp[:], op=OP.mult)
        h_f = wp.tile([128, W16], dtf)
        nvec.tensor_tensor(h_f[:], n_g[:], tmp2[:], op=OP.add)

        # hT image (aug): drop the pad columns
        nvec.tensor_copy(
            hT[:, 0:8 * BL].rearrange("p (k b) -> p k b", k=8),
            h_f[:].rearrange("p (k w) -> p k w", k=8)[:, :, 0:BL],
        )
        ngp.memset(hT[:, 8 * BL:9 * BL], 0.0)
        ngp.memset(hT[0:1, 8 * BL:9 * BL], 1.0)

    # ============================================ qpT + big attentions
    # qpT shares the attention ExitStack so wm/mem DMAs can be interleaved
    # with the wq loads (head-latency ordering: wm0, wq0, mem(utt,b0), ...).
    if KPH < 2:
        return
    K3S = int(os.environ.get("K3S", "9"))
    att_cfg = [
        ("utt", 0, LU, d["memT_utt"], w_utt, mask_t["utt"], None, None),
        ("cue", 1, S * LC, d["memT_cue"], w_cue, mask_t["cue"], d["memR_cue"], "wmr1"),
        ("goal", 2, LG, d["memT_goal"], w_goal, mask_t["goal"], d["memR_goal"], "wmr2"),
    ]
    with ExitStack() as ph:
        wp = ph.enter_context(tc.tile_pool(name="att_w", bufs=1))
        mp = ph.enter_context(tc.tile_pool(name="att_mem", bufs=3))
        thp = ph.enter_context(tc.tile_pool(name="att_th", bufs=2))
        sp = ph.enter_context(tc.tile_pool(name="att_small", bufs=2))
        jp = ph.enter_context(tc.tile_pool(name="att_junk", bufs=1))
        pj_p = ph.enter_context(tc.tile_pool(name="att_pj", bufs=4, space="PSUM"))
        sc_p = ph.enter_context(tc.tile_pool(name="att_sc", bufs=2, space="PSUM"))

        # DMA issue order tuned for the critical path into utt/b0
        mem0 = mp.tile([128, 8 * LU], dt8, tag="mem")
        with ExitStack() as qh:
            qwp = qh.enter_context(tc.tile_pool(name="qp_w", bufs=2))
            qpp = qh.enter_context(tc.tile_pool(name="qp_ps", bufs=2, space="PSUM"))
            wm_tiles = []
            wm0 = wp.tile([128, 8 * 1024], dt8, name="wm0", tag="wm0")
            nc.sync.dma_start(wm0[:], d["wm0"])
            wm_tiles.append(wm0)
            wq_tiles = []
            wq0 = qwp.tile([128, 9 * 1024], dt8, tag="wq")
            nc.sync.dma_start(wq0[:], d["wq0"])
            wq_tiles.append(wq0)
            nc.sync.dma_start(mem0[:], d["memT_utt"][0])
            for a in (1, 2):
                wq = qwp.tile([128, 9 * 1024], dt8, tag="wq")
                nc.sync.dma_start(wq[:], d[f"wq{a}"])
                wq_tiles.append(wq)
                wm = wp.tile([128, 8 * 1024], dt8, name=f"wm{a}", tag=f"wm{a}")
                nc.sync.dma_start(wm[:], d[f"wm{a}"])
                wm_tiles.append(wm)
            wq3 = qwp.tile([128, 9 * 1024], dt8, tag="wq")
            nc.sync.dma_start(wq3[:], d["wq3"])
            wq_tiles.append(wq3)

            for a in range(4):
                for m in range(8):
                    ps = qpp.tile([128, BL], dtf, tag="qp_ps")
                    for k in range(9):
                        npe.matmul(
                            ps[:],
                            wq_tiles[a][:, k * 1024 + m * 128: k * 1024 + m * 128 + 128],
                            hT[:, k * BL:(k + 1) * BL],
                            start=(k == 0),
                            stop=(k == 8),
                        )
                    nvec.tensor_scalar(
                        qpT[a][:, m * BL:(m + 1) * BL], ps[:], 1.0 / WS, None,
                        op0=OP.mult,
                    )

        wrp = ph.enter_context(tc.tile_pool(name="att_wr", bufs=2))
        wr_tiles = {}
        for name, a, L, memd, wrow, maskt, memrd, wrkey in att_cfg:
            if wrkey is not None:
                wr = wrp.tile([128, 8 * 1024], dt8, tag="wr")
                nc.sync.dma_start(wr[:], d[wrkey])
                wr_tiles[a] = wr

        for name, a, L, memd, wrow, maskt, memrd, wrkey in att_cfg:
            wm = wm_tiles[a]
            wr = wr_tiles.get(a)
            for b in range(BL):
                if a == 0 and b == 0:
                    mem = mem0
                else:
                    mem = mp.tile([128, 8 * L], dt8, tag="mem")
                    nc.sync.dma_start(mem[:], memd[b])
                if memrd is not None:
                    memr = mp.tile([128, 8 * L], dt8, tag="mem")
                    nc.sync.dma_start(memr[:], memrd[b])
                kick_w2(2)
                sc_ps = sc_p.tile([1, L], dtf, tag="sc")
                for m in range(8):
                    pj = pj_p.tile([128, L], dtf, tag="pj")
                    passes = [(wm, mem)]
                    if memrd is not None:
                        passes += [(wr, mem), (wm, memr)]
                    nmm = 4 * len(passes)
                    i = 0
                    for wsrc, msrc in passes:
                        for kp in range(4):
                            npe.matmul(
                                pj[:],
                                _pairs(wsrc[:], 8, kp, m * 128, 128),
                                _pairs(msrc[:], 8, kp, 0, L),
                                start=(i == 0),
                                stop=(i == nmm - 1),
                                perf_mode=DR,
                            )
                            i += 1
                    th = thp.tile([128, L], dtb, tag="th")
                    nsc.activation(
                        th[:], pj[:], AF.Tanh,
                        bias=qpT[a][:, m * BL + b: m * BL + b + 1],
                        scale=1.0 / WS,
                    )
                    npe.matmul(
                        sc_ps[:],
                        v_bf_t[:, a * 8 + m: a * 8 + m + 1],
                        th[:],
                        start=(m == 0),
                        stop=(m == 7),
                    )
                # scores + mask -> sbuf
                if K3S < 2:
                    continue
                sc_row = sp.tile([1, L], dtf, tag="sc_row")
                nvec.scalar_tensor_tensor(
                    sc_row[:], sc_ps[:], 1.0, maskt[:, b * L:(b + 1) * L],
                    op0=OP.mult, op1=OP.add,
                )
                ex_row = sp.tile([1, L], dtf, tag="ex_row")
                sum1 = sp.tile([1, 1], dtf, tag="sum1")
                nsc.activation(ex_row[:], sc_row[:], AF.Exp, accum_out=sum1[:])
                if name == "cue":
                    sums8 = sp.tile([1, S], dtf, tag="sums8")
                    nvec.tensor_reduce(
                        sums8[:], ex_row[:].rearrange("a (s l) -> a s l", s=S),
                        axis=mybir.AxisListType.X, op=OP.add,
                    )
                    rec8 = sp.tile([1, S], dtf, tag="rec8")
                    nvec.reciprocal(rec8[:], sums8[:])
                    kgr = sp.tile([1, S], dtf, tag="kgr")
                    nvec.tensor_tensor(
                        kgr[:], rec8[:], kg_t[:, b * S:(b + 1) * S], op=OP.mult
                    )
                    nvec.tensor_tensor(
                        wrow[:, b * L:(b + 1) * L].rearrange("a (s l) -> a s l", s=S),
                        ex_row[:].rearrange("a (s l) -> a s l", s=S),
                        kgr[:, :, None].to_broadcast([1, S, LC]),
                        op=OP.mult,
                    )
                else:
                    rec1 = sp.tile([1, 1], dtf, tag="rec1")
                    nvec.reciprocal(rec1[:], sum1[:])
                    nvec.tensor_scalar(
                        wrow[:, b * L:(b + 1) * L], ex_row[:], rec1[:], None,
                        op0=OP.mult,
                    )
                # ctx via gpsimd partition-broadcast + fused mul-reduce
                if K3S < 3:
                    continue
                wb_bf = thp.tile([128, L], dtb, tag="wb_bf")
                ngp.partition_broadcast(wb_bf[:], wrow[:, b * L:(b + 1) * L])
                if K3S < 4:
                    continue
                for k in range(8):
                    junk = jp.tile([128, L], dtb, tag="junk")
                    nvec.scalar_tensor_tensor(
                        junk[:],
                        mem[:, k * L:(k + 1) * L],
                        1.0,
                        wb_bf[:],
                        op0=OP.mult,
                        op1=OP.mult,
                        accum_out=ctxT[a][:, k * BL + b: k * BL + b + 1],
                    )

    # ============================================================ hl attn
    if KPH < 4:
        return
    with ExitStack() as ph:
        wp = ph.enter_context(tc.tile_pool(name="hl_w", bufs=1))
        sp = ph.enter_context(tc.tile_pool(name="hl_t", bufs=2))
        pp = ph.enter_context(tc.tile_pool(name="hl_ps", bufs=2, space="PSUM"))

        wmh = wp.tile([128, 8 * 1024], dt8)
        nc.sync.dma_start(wmh[:], d["wm3"])
        kick_w2(2)
        stack = wp.tile([128, 8 * 3 * BL], dtb)  # free = k*24 + b*3 + i
        for i in range(3):
            nvec.tensor_copy(
                stack[:].rearrange("p (k b i) -> p k b i", k=8, b=BL)[:, :, :, i:i + 1],
                ctxT[i][:].rearrange("p (k b) -> p k b", k=8)[:, :, :, None],
            )
        sc24_ps = pp.tile([1, 3 * BL], dtf, tag="hl_sc")
        for m in range(8):
            pj = pp.tile([128, 3 * BL], dtf, tag="hl_pj")
            for k in range(8):
                npe.matmul(
                    pj[:],
                    wmh[:, k * 1024 + m * 128: k * 1024 + m * 128 + 128],
                    stack[:, k * 24:(k + 1) * 24],
                    start=(k == 0),
                    stop=(k == 7),
                )
            pj2 = sp.tile([128, 3 * BL], dtf, tag="hl_pj2")
            nvec.scalar_tensor_tensor(
                pj2[:].rearrange("p (b i) -> p b i", b=BL),
                pj[:].rearrange("p (b i) -> p b i", b=BL),
                1.0 / WS,
                qpT[3][:, m * BL:(m + 1) * BL][:, :, None].to_broadcast([128, BL, 3]),
                op0=OP.mult, op1=OP.add,
            )
            th = sp.tile([128, 3 * BL], dtb, tag="hl_th")
            nsc.activation(th[:], pj2[:], AF.Tanh)
            npe.matmul(
                sc24_ps[:], v_img_t[:, 24 + m: 24 + m + 1], th[:],
                start=(m == 0), stop=(m == 7),
            )
        ex24 = sp.tile([1, 3 * BL], dtf, tag="hl_ex")
        nsc.activation(ex24[:], sc24_ps[:], AF.Exp, scale=1.0 / VS)
        sums = sp.tile([1, BL], dtf, tag="hl_sums")
        nvec.tensor_reduce(
            sums[:], ex24[:].rearrange("a (b i) -> a b i", b=BL),
            axis=mybir.AxisListType.X, op=OP.add,
        )
        recs = sp.tile([1, BL], dtf, tag="hl_rec")
        nvec.reciprocal(recs[:], sums[:])
        nvec.tensor_tensor(
            ctw[:].rearrange("a (b i) -> a b i", b=BL),
            ex24[:].rearrange("a (b i) -> a b i", b=BL),
            recs[:, :, None].to_broadcast([1, BL, 3]),
            op=OP.mult,
        )
        # ctx_merge
        wb24 = sp.tile([128, 3 * BL], dtf, tag="hl_wb_sb")
        ngp.partition_broadcast(wb24[:], ctw[:])
        for k in range(8):
            prod = sp.tile([128, 3 * BL], dtf, tag="hl_prod")
            nvec.tensor_tensor(prod[:], stack[:, k * 24:(k + 1) * 24], wb24[:], op=OP.mult)
            nvec.tensor_reduce(
                cmT[:, k * BL:(k + 1) * BL],
                prod[:].rearrange("p (b i) -> p b i", b=BL),
                axis=mybir.AxisListType.X, op=OP.add,
            )

    # ======================================================= pgen + merge
    if KPH < 5:
        return
    with ExitStack() as ph:
        wp = ph.enter_context(tc.tile_pool(name="pg_w", bufs=1))
        sp = ph.enter_context(tc.tile_pool(name="pg_t", bufs=2))
        pp = ph.enter_context(tc.tile_pool(name="pg_ps", bufs=2, space="PSUM"))

        pgt = wp.tile([128, 21 * BL], dtb)  # pgen_in image, kt 4..20 = out_in
        ngp.memset(pgt[:], 0.0)
        nc.sync.dma_start(pgt[:, 0:4 * BL], d["embT"])
        nvec.tensor_copy(pgt[:, 4 * BL:12 * BL], hT[:, 0:8 * BL])
        cmT_bf = sp.tile([128, 8 * BL], dtb, tag="cm_bf")
        nvec.tensor_copy(cmT_bf[:], cmT[:])
        nvec.tensor_copy(pgt[:, 12 * BL:20 * BL], cmT_bf[:])
        ngp.memset(pgt[0:1, 20 * BL:21 * BL], 1.0)

        pgw = wp.tile([128, 21], dtb)
        nc.sync.dma_start(pgw[:], d["pgenT"])
        kick_w2(2)
        ps = pp.tile([1, BL], dtf, tag="pg_ps")
        for k in range(21):
            npe.matmul(
                ps[:], pgw[:, k:k + 1], pgt[:, k * BL:(k + 1) * BL],
                start=(k == 0), stop=(k == 20),
            )
        pg_t = sp.tile([1, BL], dtf, tag="pg_tanh")
        nsc.activation(pg_t[:], ps[:], AF.Tanh, scale=0.5)
        nvec.tensor_scalar(pgen_row[:], pg_t[:], 0.5, 0.5, op0=OP.mult, op1=OP.add)
        nvec.tensor_scalar(ompg_row[:], pg_t[:], -0.5, 0.5, op0=OP.mult, op1=OP.add)

        # gates = ct_w * (1 - pgen)   [1, 24] (b, i)
        gates = sp.tile([1, 3 * BL], dtf, tag="gates")
        nvec.tensor_tensor(
            gates[:].rearrange("a (b i) -> a b i", b=BL),
            ctw[:].rearrange("a (b i) -> a b i", b=BL),
            ompg_row[:, :, None].to_broadcast([1, BL, 3]),
            op=OP.mult,
        )

        # ------------------------------------------------ out1 -> out_mid
        o1w = wp.tile([128, 17 * 1024], dt8)
        nc.sync.dma_start(o1w[:], d["out1T"])
        omid8 = sp.tile([128, 8 * BL], dtb, tag="omid8")
        for m in range(8):
            mps = pp.tile([128, BL], dtf, tag="pg_om")
            for k in range(17):
                npe.matmul(
                    mps[:],
                    o1w[:, k * 1024 + m * 128: k * 1024 + m * 128 + 128],
                    pgt[:, (4 + k) * BL:(5 + k) * BL],
                    start=(k == 0), stop=(k == 16),
                )
            nvec.tensor_scalar(
                omid8[:, m * BL:(m + 1) * BL], mps[:], 1.0 / WS, None, op0=OP.mult
            )
        # rep-16 fp8 image [128, 8*128], col = k*128 + r*8 + b  (persist tile)
        nvec.tensor_copy(
            omid32[:].rearrange("p (k r b) -> p k r b", k=8, r=16),
            omid8[:].rearrange("p (k b) -> p k b", k=8)[:, :, None, :]
            .to_broadcast([128, 8, 16, BL]),
        )

        # attn_merge rows (bf16) -> attnT image via PE transposes
        for b in range(BL):
            arow = sp.tile([1, NJ], dtb, tag="arow")
            nvec.tensor_scalar(
                arow[:, 0:LU], w_utt[:, b * LU:(b + 1) * LU],
                gates[:, 3 * b: 3 * b + 1], None, op0=OP.mult,
            )
            nvec.tensor_scalar(
                arow[:, LU:LU + S * LC], w_cue[:, b * S * LC:(b + 1) * S * LC],
                gates[:, 3 * b + 1: 3 * b + 2], None, op0=OP.mult,
            )
            nvec.tensor_scalar(
                arow[:, LU + S * LC:NJ_REAL], w_goal[:, b * LG:(b + 1) * LG],
                gates[:, 3 * b + 2: 3 * b + 3], None, op0=OP.mult,
            )
            ngp.memset(arow[:, NJ_REAL:NJ], 0.0)
            for k in range(NKJ):
                tp = pp.tile([128, 1], dtb, tag="pg_tp")
                npe.transpose(tp[:], arow[:, k * 128:(k + 1) * 128], id_bf[0:1, 0:1])
                nvec.tensor_copy(attnT[:, k * BL + b: k * BL + b + 1], tp[:])

        # lo/hi one-hot scatter inputs (scatter itself overlaps out2 below)
        nc.sync.dma_start(lo_t[:], d["lo_img"])
        nc.sync.dma_start(hi_t[:], d["hi_img"])

    # =================================================== out2 + softmax
    if KPH < 6:
        return
    logits_dram_pool = ctx.enter_context(
        tc.tile_pool(name="ldram", bufs=1, space="DRAM")
    )
    logits_dram = logits_dram_pool.tile([BL, VP], dt16)

    with ExitStack() as ph:
        lg = ph.enter_context(tc.tile_pool(name="o2_lg", bufs=2))
        sp = ph.enter_context(tc.tile_pool(name="o2_t", bufs=4))
        pp = ph.enter_context(tc.tile_pool(name="o2_ps", bufs=3, space="PSUM"))
        gp_ps = ph.enter_context(tc.tile_pool(name="o2_gps", bufs=2, space="PSUM"))

        QC = NVC // 4  # chunks per lg quarter
        lg8 = None
        for cidx in range(NVC):
            if cidx % QC == 0:
                lg8 = lg.tile([8, QC * 512], dt16, tag="lg8")
            if cidx not in o2_tiles:
                kick_w2(1, jit=True)
            o2 = o2_tiles[cidx]
            ps = pp.tile([128, 512], dtf, tag="o2_ps")
            for kp in range(4):
                npe.matmul(
                    ps[:],
                    _pairs(omid32[:], 8, kp, 0, 128),
                    _pairs(o2[:], 8, kp, 0, 512),
                    start=(kp == 0), stop=(kp == 3),
                    perf_mode=DR,
                )
            nsc.activation(
                lg8[:, (cidx % QC) * 512:(cidx % QC + 1) * 512], ps[0:8, :], AF.Copy
            )
            if cidx % QC == QC - 1:
                q = cidx // QC
                nc.sync.dma_start(
                    logits_dram[:][:, q * QC * 512:(q + 1) * QC * 512], lg8[:]
                )
            # interleaved pointer scatter: one local batch row per 8 chunks
            if cidx % 8 == 7:
                b = cidx // 8
                gps = gp_ps.tile([128, 256], dtf, tag="pg_grid")
                for k in range(NKJ):
                    mm = sp.tile([128, 128], dtb, tag="mm")
                    nvec.tensor_scalar(
                        mm[:], iota128[:], hi_t[:, k * BL + b: k * BL + b + 1], None,
                        op0=OP.is_equal,
                    )
                    nt = sp.tile([128, 256], dtb, tag="nt")
                    nvec.scalar_tensor_tensor(
                        nt[:], iota256[:], lo_t[:, k * BL + b: k * BL + b + 1],
                        attnT[:, k * BL + b: k * BL + b + 1].to_broadcast([128, 256]),
                        op0=OP.is_equal, op1=OP.mult,
                    )
                    npe.matmul(gps[:], mm[:], nt[:], start=(k == 0), stop=(k == NKJ - 1))
                nvec.tensor_copy(scat_sb[:, b * 256:(b + 1) * 256], gps[:])

    if KPH < 7:
        return
    with ExitStack() as ph:
        gp = ph.enter_context(tc.tile_pool(name="fin", bufs=1))
        pp = ph.enter_context(tc.tile_pool(name="fin_ps", bufs=2, space="PSUM"))
        sp = ph.enter_context(tc.tile_pool(name="fin_t", bufs=2))

        grid16 = gp.tile([128, BL * 256], dt16)
        src = bass.AP(
            logits_dram[:].tensor, 0,
            [[256, 128], [VP, BL], [1, 256]],
        )
        nc.sync.dma_start(grid16[:].rearrange("p (b f) -> p b f", b=BL), src)
        bias_g = gp.tile([128, 256], dtf)
        nc.sync.dma_start(bias_g[:], d["w2b"])
        ex = gp.tile([128, BL * 256], dtf)
        colsums = gp.tile([128, BL], dtf)
        grid = gp.tile([128, BL * 256], dtf)
        for b in range(BL):
            nvec.scalar_tensor_tensor(
                grid[:, b * 256:(b + 1) * 256],
                grid16[:, b * 256:(b + 1) * 256], 1.0 / WS, bias_g[:],
                op0=OP.mult, op1=OP.add,
            )
            nsc.activation(
                ex[:, b * 256:(b + 1) * 256], grid[:, b * 256:(b + 1) * 256],
                AF.Exp, accum_out=colsums[:, b:b + 1],
            )
        den_ps = pp.tile([BL, 1], dtf, tag="fin_den")
        npe.matmul(den_ps[:], colsums[:], ones_c_f[:], start=True, stop=True)
        den_sb = sp.tile([BL, 1], dtf, tag="fin_densb")
        nvec.tensor_copy(den_sb[:], den_ps[:])
        denT_ps = pp.tile([1, BL], dtf, tag="fin_denT")
        npe.transpose(denT_ps[:], den_sb[:], id_f32[0:BL, 0:BL])
        rec_row = sp.tile([1, BL], dtf, tag="fin_rec")
        nvec.reciprocal(rec_row[:], denT_ps[:])
        scal_row = sp.tile([1, BL], dtf, tag="fin_scal")
        nvec.tensor_tensor(scal_row[:], rec_row[:], pgen_row[:], op=OP.mult)
        scb = sp.tile([128, BL], dtf, tag="fin_scbsb")
        ngp.partition_broadcast(scb[:], scal_row[:])
        final = gp.tile([128, BL * 256], dtf)
        for b in range(BL):
            nvec.scalar_tensor_tensor(
                final[:, b * 256:(b + 1) * 256],
                ex[:, b * 256:(b + 1) * 256],
                scb[:, b:b + 1],
                scat_sb[:, b * 256:(b + 1) * 256],
                op0=OP.mult, op1=OP.add,
            )
        dst = bass.AP(d["outp"].tensor, 0, [[256, 128], [VP, BL], [1, 256]])
        nc.sync.dma_start(dst, final[:].rearrange("p (b f) -> p b f", b=BL))


def _get_program():
    if "nc" in _CACHE:
        return _CACHE["nc"]
    from contextlib import ExitStack

    nc = bacc.Bacc(
        "TRN2", target_bir_lowering=False, debug=False,
        enable_asserts=False, num_devices=NCORES,
    )
    d = _declare(nc)
    with tile.TileContext(nc, pool_alloc_mode="queue") as tc:
        with ExitStack() as ctx:
            _build(nc, tc, ctx, d)
    nc.compile()
    _CACHE["nc"] = nc
    return nc


def kernel(**inputs):
    global LAST_RESULTS
    in_maps = _prep_inputs(inputs)
    nc = _get_program()
    os.environ["BASS_NEVER_TRACE"] = "1"  # no NTFF hook in this container
    res = run_bass_kernel_spmd(nc, in_maps, core_ids=list(range(NCORES)))
    LAST_RESULTS = res
    out = np.concatenate([res.results[c]["outp"] for c in range(NCORES)], axis=0)
    return np.ascontiguousarray(out[:, :VEXT]).astype(np.float32)
